# revision 86
# baseline (speedup 1.0000x reference)
"""DCN CrossLayer kernel for Trainium2 (8 NeuronCores, batch-sharded).

Math: the reference loop
    cross = x
    for i in range(L):
        s_i   = sum(cross, axis=1)                  # (B, 1)
        cross = s_i * x * W[i] + b[i] + cross
collapses to
    out[b, k] = x[b, k] * (1 + sum_i s_i[b] * W[i, k]) + Bsum[k]
with
    u_i[b]  = sum_k x[b, k] * W[i, k]
    s_0[b]  = sum_k x[b, k]
    s_{i+1} = s_i * (1 + u_i) + beta_i,   beta_i = sum_k b[i, k]
    Bsum[k] = sum_i b[i, k]

Device work per 128-row tile: PE transposes x (the contraction must run on
partitions), one accumulated matmul x @ [ones,W0,W1,W2] -> [s0,u0,u1,u2],
a 4-step recursion on DVE, one matmul S' @ [W;ones] -> T, and a single
elementwise multiply out = x * T.  b is all-zeros for this problem; if a
caller passes b != 0, beta_i is compiled into the recursion and Bsum is
added on the host after the gather.

Precision modes:
  "f32r"  — fp32r (tf32-like, ~11-bit mantissa) matmuls; rel err ~1.4e-4.
  "exact" — fp32 U-matmul (exact row reductions); only the small S'@W''
            matmul stays fp32r, so rel err drops to ~9.6e-5 at ~40% more
            time (PE-bound).  (A fully-compensated V via K-stacked
            [s_r; s_e] bands was prototyped but walrus rejects matmul
            PSUM dst partition bases other than 0.)
"""

import sys

sys.path.insert(0, "/opt/trn_rl_repo")

import numpy as np

import concourse.bacc as bacc
import concourse.tile as tile
from concourse import mybir
from concourse.bass_utils import run_bass_kernel_spmd
from concourse.masks import make_identity

N_CORES = 8
B, D, L = 8192, 2048, 4
ROWS = B // N_CORES          # 1024 rows per core
P = 128                      # partitions
TILES = ROWS // P            # 8 tiles per core
PAIRS = TILES // 2           # tiles processed in pairs (N=256 matmuls)
KC = D // P                  # 16 k-chunks per row-tile
NT = D // 1024               # 2 psum T-chunks (2 banks each) per tile

F32 = mybir.dt.float32
F32R = mybir.dt.float32r
ADD = mybir.AluOpType.add
MULT = mybir.AluOpType.mult

# "f32r": ~54 us/core modeled, rel err ~1.4e-4 (tf32-class).
# "exact": ~77 us/core modeled, rel err ~9.6e-5 (PE-bound fp32 U-matmul).
PRECISION = "f32r"


def build_program(betas, precision=PRECISION):
    """Build the per-core Bass program (same program on all 8 cores)."""
    exact = precision == "exact"
    nc = bacc.Bacc("TRN2", target_bir_lowering=False)

    x_d = nc.dram_tensor("x", [ROWS, D], F32, kind="ExternalInput")
    a_d = nc.dram_tensor("acoef", [P, KC, L], F32, kind="ExternalInput")
    wv_d = nc.dram_tensor("wv", [L + 1, D], F32, kind="ExternalInput")
    out_d = nc.dram_tensor("out", [ROWS, D], F32, kind="ExternalOutput")

    # x lives in one persistent SBUF tile. Loads are pair-granular (2 MiB)
    # except the first pair, which splits into two 1 MiB DMAs so the PE can
    # start transposing ~4 us earlier; stores are pair-granular (the tail is
    # compute-paced, so each store should go as soon as its pair is done).
    x_t = x_d.rearrange("(t p) m -> p t m", p=P)
    out_t = out_d.rearrange("(t p) m -> p t m", p=P)
    LOAD_SPLITS = ((0, 1), (1, 2), (2, 4), (4, 6), (6, 8))
    STORE_SPLITS = ((0, 2), (2, 4), (4, 6), (6, 8))

    xt_dt = F32 if exact else F32R
    # Transposes stay fp32: an fp32r transpose would force the x tile itself
    # to be fp32r-rounded (verifier checks producers transitively), which
    # would corrupt the final out = x * T multiply.
    tr_dt = F32

    with tile.TileContext(nc) as tc:
        with (
            tc.tile_pool(name="consts", bufs=1) as consts,
            tc.tile_pool(name="xp", bufs=1) as xp,
            tc.tile_pool(name="xtp", bufs=2) as xtp,
            tc.tile_pool(name="smalls", bufs=4) as smalls,
            tc.tile_pool(name="tr_ps", bufs=2, space="PSUM") as tr_ps,
            tc.tile_pool(name="ut_ps", bufs=2, space="PSUM") as ut_ps,
            tc.tile_pool(name="stage_ps", bufs=1, space="PSUM") as stage_ps,
            tc.tile_pool(name="t_ps", bufs=3, space="PSUM") as t_ps,
        ):
            # Data loads go ahead of everything so the DMA engines start on
            # the critical 16 MiB stream immediately; consts ride SWDGE.
            xall = xp.tile([P, TILES, D], F32, tag="x")
            for i, (lo, hi) in enumerate(LOAD_SPLITS):
                nc.sync.dma_start(out=xall[:, lo:hi, :], in_=x_t[:, lo:hi, :])
                if i == 0:
                    ident = consts.tile([P, P], F32)
                    make_identity(nc, ident)
                    a_sb = consts.tile([P, KC, L], F32)
                    nc.gpsimd.dma_start(out=a_sb, in_=a_d[:])
                    wv_sb = consts.tile([L + 1, D], F32)
                    nc.gpsimd.dma_start(out=wv_sb, in_=wv_d[:])

            if exact:
                a_use = a_sb  # fp32 U-matmul, no rounding
            else:
                # fp32r matmul operands must be written fp32r-rounded.
                a_use = consts.tile([P, KC, L], F32R)
                nc.any.tensor_copy(a_use, a_sb)
            wv_use = consts.tile([L + 1, D], F32R)
            nc.any.tensor_copy(wv_use, wv_sb)

            def front_half(g):
                """Transposes + U-matmul accumulation for pair g."""
                xs = [xall[:, 2 * g, :], xall[:, 2 * g + 1, :]]
                xt = xtp.tile([P, KC, 2 * P], xt_dt)
                ut = ut_ps.tile([L, 2 * P], F32, tag="ut")
                # 4 [128,128] transpose blocks per PSUM bank, one [128,512]
                # copy back to SBUF per bank.  U^T = A^T @ x^T accumulates
                # with one group of lag so the PE never waits on the copy it
                # consumes; interleaving also keeps HAM-relevant matmul
                # activity dense on real HW (transposes don't tick HAM).
                # (row 0 = s_0 via the ones column of A, rows 1..3 = u_i.)
                for j in range(KC // 2):
                    ps = tr_ps.tile([P, 4 * P], tr_dt)
                    for idx, (c, t) in enumerate(
                        [(2 * j, 0), (2 * j, 1), (2 * j + 1, 0), (2 * j + 1, 1)]
                    ):
                        nc.tensor.transpose(
                            ps[:, idx * P : (idx + 1) * P],
                            xs[t][:, c * P : (c + 1) * P],
                            ident,
                        )
                    nc.any.tensor_copy(xt[:, 2 * j : 2 * j + 2, :], ps.bitcast(F32))
                for c in range(KC):
                    nc.tensor.matmul(
                        ut,
                        a_use[:, c, :],
                        xt[:, c, :],
                        start=(c == 0),
                        stop=(c == KC - 1),
                    )
                return xs, ut

            def back_half(g, xs, ut):
                """Recursion, V-matmul, final multiply and store for pair g."""
                # Compute-engine operands must start at partition 0 (mod 32),
                # so run the tiny recursion in natural layout: transpose
                # U^T -> [row, coeff], recurse column-wise, transpose back.
                ut_sb = smalls.tile([L, 2 * P], F32, tag="ut_sb")
                nc.scalar.copy(ut_sb, ut)
                # un (natural U) and the S'^T staging share one PSUM bank:
                # un at free cols 0..7, st bands at cols 8..263.
                stage = stage_ps.tile([P, 512], F32, tag="stage")
                un = stage[:, 0:8].rearrange("p (h l) -> p h l", h=2)
                for h in range(2):
                    nc.tensor.transpose(
                        un[:, h, :], ut_sb[:, h * P : (h + 1) * P], ident[:L, :L]
                    )

                # sn columns per half: [s_0, s_1, s_2, s_3, 1]
                sn = smalls.tile([P, 2, L + 1], F32, tag="sn")
                nc.gpsimd.memset(sn[:, :, L], 1.0)
                nc.vector.tensor_copy(sn[:, :, 0], un[:, :, 0])
                for i in range(L - 1):
                    nc.vector.scalar_tensor_tensor(
                        out=sn[:, :, i + 1],
                        in0=un[:, :, i + 1],
                        scalar=1.0,
                        in1=sn[:, :, i],
                        op0=ADD,
                        op1=MULT,
                    )
                    if betas[i] != 0.0:
                        nc.vector.tensor_scalar_add(
                            sn[:, :, i + 1], sn[:, :, i + 1], float(betas[i])
                        )

                # S'^T via transpose back; the copy rounds to fp32r.
                st_ps = stage[0 : L + 1, 8 : 8 + 2 * P]
                for h in range(2):
                    nc.tensor.transpose(
                        st_ps[:, h * P : (h + 1) * P], sn[:, h, :], ident
                    )
                st = smalls.tile([L + 1, 2 * P], F32R, tag="st")
                nc.scalar.copy(st, st_ps)

                # T = S' @ [W; ones] per 512-wide chunk, then out = x * T.
                # The last pair's multiply chain is the kernel tail (DVE is
                # the only engine that can read PSUM for tensor_tensor), so
                # for it route two chunks per tile via an ACT PSUM->SBUF copy
                # + GPSIMD multiply, halving the DVE tail chain.
                last = g >= PAIRS - 2
                for t in range(2):
                    for ch in range(D // 512):
                        tp = t_ps.tile([P, 512], F32)
                        nn = ch * 512
                        nc.tensor.matmul(
                            tp,
                            st[:, t * P : (t + 1) * P],
                            wv_use[:, nn : nn + 512],
                            start=True,
                            stop=True,
                        )
                        sl = slice(nn, nn + 512)
                        if last and ch % 2 == 1:
                            tsb = smalls.tile([P, 512], F32, tag="tsb")
                            nc.scalar.copy(tsb, tp)
                            nc.gpsimd.tensor_mul(xs[t][:, sl], xs[t][:, sl], tsb)
                        else:
                            nc.any.tensor_mul(xs[t][:, sl], xs[t][:, sl], tp)
                    # Issue each store as soon as the last tile it covers
                    # has its multiplies queued.
                    tile_idx = 2 * g + t
                    for lo, hi in STORE_SPLITS:
                        if tile_idx == hi - 1:
                            nc.sync.dma_start(
                                out=out_t[:, lo:hi, :], in_=xall[:, lo:hi, :]
                            )

            # Emit each pair's front and back halves in order; Tile's static
            # scheduler interleaves across pairs better than a manual
            # two-stage software pipeline (measured: manual pipelining
            # inverts priorities on ACT/DVE and delays stores by ~2 us).
            for g in range(PAIRS):
                xs, ut = front_half(g)
                back_half(g, xs, ut)

    nc.finalize()
    return nc


_CACHE = {}


def _get_program(betas, precision=PRECISION):
    key = (tuple(float(b) for b in betas), precision)
    if key not in _CACHE:
        _CACHE[key] = build_program(key[0], precision)
    return _CACHE[key]


def make_in_maps(x, W, b):
    """Shard x across cores; replicate the tiny coefficient tensors."""
    x = np.ascontiguousarray(np.asarray(x, dtype=np.float32))
    W = np.asarray(W, dtype=np.float32)
    assert x.shape == (B, D) and W.shape == (L, D)

    # A = [ones, W0, W1, W2] as [128, KC, L]: A_sb[p, c, m] = A[c*128+p, m]
    a_mat = np.concatenate([np.ones((D, 1), np.float32), W[: L - 1].T], axis=1)
    a_host = np.ascontiguousarray(
        a_mat.reshape(KC, P, L).transpose(1, 0, 2).astype(np.float32)
    )
    # W'' = [W; ones] as [L+1, D]
    wv_host = np.ascontiguousarray(
        np.concatenate([W, np.ones((1, D), np.float32)], axis=0)
    )
    shards = x.reshape(N_CORES, ROWS, D)
    return [
        {"x": shards[i], "acoef": a_host, "wv": wv_host} for i in range(N_CORES)
    ]


def kernel(**inputs) -> np.ndarray:
    x = np.asarray(inputs["x"], dtype=np.float32)
    W = np.asarray(inputs["W"], dtype=np.float32)
    b = np.asarray(inputs["b"], dtype=np.float32)

    betas = b.sum(axis=1, dtype=np.float64).astype(np.float32)
    nc = _get_program(betas)
    in_maps = make_in_maps(x, W, b)
    res = run_bass_kernel_spmd(nc, in_maps, list(range(N_CORES)))
    out = np.concatenate([res.results[i]["out"] for i in range(N_CORES)], axis=0)

    bsum = b.sum(axis=0, dtype=np.float64).astype(np.float32)
    if np.any(bsum != 0.0):
        out = out + bsum[None, :]
    return out
